# revision 2
# baseline (speedup 1.0000x reference)
"""LCNN conv2d kernel for Trainium2 (8 NeuronCores, batch-sharded).

Math: out[b,o,h,w] = sum_d Wmat[o,d] * conv2d(x, dictionary)[b,d,h,w]
where Wmat is the scatter-add of lookup_coefficients into [O, D].

Device strategy (per core, 2 batches), all-bf16:
 - single input buffer XX [128, F+PW]: rows 0:64 = padded x, rows 64:128 =
   x shifted by (1 row + 1 col).  This supports 3 diagonal tap pairs
   {(0,0),(1,1)} {(0,1),(1,2)} {(1,0),(2,1)} as K=128 matmuls, and the 3
   leftover taps (0,2) (2,0) (2,2) as K=64 row-group matmuls that run
   pairwise-concurrently in the PE array (tile t0 on rows 0:64, t1 on
   rows 64:128).  Same 4.5 PE slots/tile as 4-way duplication, but half
   the input HBM traffic (4.9 MB vs 9.8 MB per core).
 - stage 2: [O=256, D->128] channel-mix as 2 matmuls per tile on the bf16
   copy of the conv PSUM tile.
 - outputs staged as bf16 and DMA'd once per tile-pair (393 KB per
   trigger; b=0 pairs on the gpsimd SWDGE ring, b=1 on the sync HWDGE
   ring); host upcasts to f32.
 - input loaded in 7 growing chunks (12..42 rows) to minimize
   time-to-first-matmul while keeping DMA trigger count low.
"""
import os
import sys

for _p in ("/opt/trn_rl_repo", "/root/.axon_site/_ro/trn_rl_repo"):
    if os.path.isdir(_p) and _p not in sys.path:
        sys.path.insert(0, _p)

import ml_dtypes
import numpy as np
from contextlib import ExitStack

from concourse import bacc, mybir, tile
from concourse.bass_utils import run_bass_kernel_spmd

# problem shapes (hardcoded per contract)
B, CIN, H, W = 16, 64, 96, 96
D, O = 100, 256
DP = 128                   # D padded to full PE width
NCORES = 8
BPC = B // NCORES          # batches per core
PH, PW = H + 2, W + 2      # zero-padded spatial
F = BPC * PH * PW          # per-partition x extent
FX = F + PW                # + tail pad so the (-1 col) view stays in-bounds
R = 4                      # output rows per matmul tile
NT = H // R                # h-tiles per batch
N = R * W                  # matmul free size (384)
PB = 512                   # psum bank stride (f32 elems)
bf16 = mybir.dt.bfloat16
f32 = mybir.dt.float32

_NC_CACHE = {}


def _build():
    nc = bacc.Bacc(None, target_bir_lowering=False, debug=False)
    xx = nc.declare_dram_parameter("xx", [128, FX], bf16, isOutput=False)
    wst = nc.declare_dram_parameter("wst", [128, 6 * DP], bf16, isOutput=False)
    wm = nc.declare_dram_parameter("wm", [DP, O], bf16, isOutput=False)
    out = nc.declare_dram_parameter("out", [BPC, O, H, W], bf16, isOutput=True)

    with tile.TileContext(nc) as tc, ExitStack() as ctx:
        sb = ctx.enter_context(tc.tile_pool(name="sb", bufs=1))
        c1p = ctx.enter_context(tc.tile_pool(name="c1p", bufs=5))
        stgp = ctx.enter_context(tc.tile_pool(name="stgp", bufs=6))
        pcp = ctx.enter_context(tc.tile_pool(name="pcp", bufs=2, space="PSUM"))
        pop = ctx.enter_context(tc.tile_pool(name="pop", bufs=2, space="PSUM"))

        XX = sb.tile([128, FX], bf16)
        wst_s = sb.tile([128, 6 * DP], bf16)
        wm_s = sb.tile([DP, O], bf16)
        # weights via gpsimd SWDGE so the sync ring's head slot goes to the
        # first x chunks; both land well before the first real matmul
        nc.gpsimd.dma_start(wst_s[:], wst[:])
        nc.gpsimd.dma_start(wm_s[:], wm[:])

        # x chunk loads on the sync HWDGE ring; small leading chunks
        # minimize time-to-first-matmul, large trailing ones minimize
        # trigger-issue cost (~0.6us of sync-engine time per trigger).
        rows = [12, 12, 20, 28, 42, 42, 41]
        assert sum(rows) * PW == FX
        a = 0
        for nr in rows:
            L = nr * PW
            nc.sync.dma_start(XX[:, a:a + L], xx[:, a:a + L])
            a += L

        # base view and the (-1 col) view used by tile t1's (2,0) tap
        XV = XX[:, 0:F].rearrange("p (b h w) -> p b h w", b=BPC, h=PH, w=PW)
        XM = XX[:, PW - 1:PW - 1 + F].rearrange(
            "p (b h w) -> p b h w", b=BPC, h=PH, w=PW)

        # PE warm-up: dummy matmuls on a zeroed SBUF tile run while the x
        # chunks stream in, paying part of the HAM ramp during otherwise
        # idle time.  They write the first pair's conv PSUM tile, which
        # the real group resets.
        warm = sb.tile([128, 256], bf16)
        nc.vector.memset(warm[:], 0)
        wq = pcp.tile([128, 2 * PB], f32, name="pcq")
        for _ in range(6):
            nc.tensor.matmul(wq[:, 0:256], warm[:, 0:128], warm[:],
                             start=True, stop=True, skip_group_check=True)
        state = {"warmq": wq}

        def stage1_pair(b, t0):
            """Two tiles' conv groups: 3 K=64 single-tap matmuls each
            (t0 on PE rows 0:64, t1 on rows 64:128 -> pairwise concurrent)
            then 3 K=128 diagonal-pair matmuls each."""
            t1 = t0 + 1
            h0, h1 = t0 * R, t1 * R
            pcq = state.pop("warmq", None)
            if pcq is None:
                pcq = pcp.tile([128, 2 * PB], f32, name="pcq")
            pcqv = pcq.rearrange("p (u n) -> p u n", u=2)
            pc0 = pcqv[:, 0, 0:N]
            pc1 = pcqv[:, 1, 0:N]
            # singles: (0,2), (2,0), (2,2); t0 via plain rows, t1 via the
            # diag-shifted rows (offsets shifted by (-1,-1))
            nc.tensor.matmul(pc0, wst_s[0:64, 3 * DP:4 * DP],
                             XV[0:64, b, h0:h0 + R, 2:2 + W],
                             start=True, stop=False)
            nc.tensor.matmul(pc1, wst_s[64:128, 3 * DP:4 * DP],
                             XV[64:128, b, h1 - 1:h1 - 1 + R, 1:1 + W],
                             start=True, stop=False)
            nc.tensor.matmul(pc0, wst_s[0:64, 4 * DP:5 * DP],
                             XV[0:64, b, h0 + 2:h0 + 2 + R, 0:W],
                             start=False, stop=False)
            nc.tensor.matmul(pc1, wst_s[64:128, 4 * DP:5 * DP],
                             XM[64:128, b, h1:h1 + R, 0:W],
                             start=False, stop=False)
            nc.tensor.matmul(pc0, wst_s[0:64, 5 * DP:6 * DP],
                             XV[0:64, b, h0 + 2:h0 + 2 + R, 2:2 + W],
                             start=False, stop=False)
            nc.tensor.matmul(pc1, wst_s[64:128, 5 * DP:6 * DP],
                             XV[64:128, b, h1 + 1:h1 + 1 + R, 1:1 + W],
                             start=False, stop=False)
            # diagonal pairs on the full K=128 array
            for t, pc, hh in ((t0, pc0, h0), (t1, pc1, h1)):
                nc.tensor.matmul(pc, wst_s[:, 0 * DP:1 * DP],
                                 XV[:, b, hh:hh + R, 0:W],
                                 start=False, stop=False)
                nc.tensor.matmul(pc, wst_s[:, 1 * DP:2 * DP],
                                 XV[:, b, hh:hh + R, 1:1 + W],
                                 start=False, stop=False)
                nc.tensor.matmul(pc, wst_s[:, 2 * DP:3 * DP],
                                 XV[:, b, hh + 1:hh + 1 + R, 0:W],
                                 start=False, stop=True)
            # one strided copy evacuates both tiles' conv PSUM banks
            c1q = c1p.tile([128, 2 * N], bf16, name="c1q")
            if (t0 // 2) % 2 == 0:
                nc.vector.tensor_copy(
                    c1q.rearrange("p (u n) -> p u n", u=2), pcqv[:, :, 0:N])
            else:
                nc.scalar.copy(
                    c1q.rearrange("p (u n) -> p u n", u=2), pcqv[:, :, 0:N])
            state[(b, t0)] = c1q[:, 0:N]
            state[(b, t1)] = c1q[:, N:2 * N]

        def stage2(b, t):
            """[O,D] channel mix for tile t; results staged into the
            pair's shared output buffer, DMA'd once per pair."""
            c1 = state.pop((b, t))
            po = pop.tile([128, 2 * PB], f32, name="po")
            pov = po.rearrange("p (u n) -> p u n", u=2)
            nc.tensor.matmul(pov[:, 0, 0:N], wm_s[:, 0:128], c1,
                             start=True, stop=True)
            nc.tensor.matmul(pov[:, 1, 0:N], wm_s[:, 128:256], c1,
                             start=True, stop=True)
            if t % 2 == 0:
                stg = stgp.tile([128, 4 * N], bf16, name="stg")
                state[("stg", b)] = stg
            else:
                stg = state[("stg", b)]
            stgv = stg.rearrange("p (u m) -> p u m", u=2)
            dstv = stgv[:, :, (t % 2) * N:(t % 2) * N + N]
            # single strided copy evacuates both 128-channel halves;
            # opposite engine from the same iteration's c1 copy
            if t % 2 == 0:
                nc.scalar.copy(dstv, pov[:, :, 0:N])
            else:
                nc.vector.tensor_copy(dstv, pov[:, :, 0:N])
            if t % 2 == 1:
                # per-pair DMA: partition o carries channels {o, 128+o},
                # each half a contiguous 1536 B run
                dst = out[b].rearrange("(u o) h w -> o u (h w)", u=2)[
                    :, :, (t - 1) * N:(t + 1) * N]
                if b == 0:
                    nc.gpsimd.dma_start(dst, stgv)
                else:
                    nc.sync.dma_start(dst, stgv)

        NP = NT // 2
        PLAG = 2    # pairs of lag -> stage2 trails by 2*PLAG tiles
        for b in range(BPC):
            for p in range(NP):
                stage1_pair(b, 2 * p)
                if p >= PLAG:
                    stage2(b, 2 * (p - PLAG))
                    stage2(b, 2 * (p - PLAG) + 1)
            for t in range(NT - 2 * PLAG, NT):
                stage2(b, t)

    nc.compile()
    return nc


def _get_nc():
    if "nc" not in _NC_CACHE:
        _NC_CACHE["nc"] = _build()
    return _NC_CACHE["nc"]


def _prep_inputs(x, dictionary, lookup_coefficients, lookup_indices):
    x = np.asarray(x, dtype=np.float32)
    dic = np.asarray(dictionary, dtype=np.float32)
    coeff = np.asarray(lookup_coefficients, dtype=np.float32).reshape(O, -1)
    idx = np.asarray(lookup_indices).astype(np.int64).reshape(O, -1)

    wmat = np.zeros((O, D), np.float32)
    np.add.at(wmat, (np.arange(O)[:, None], idx), coeff)
    wmp = np.zeros((DP, O), np.float32)
    wmp[:D] = wmat.T
    wmp = wmp.astype(ml_dtypes.bfloat16)

    # stationary slabs [128, 6*DP]: 3 diagonal pairs (lower rows = first
    # tap, upper rows = tap shifted (+1,+1)), then the 3 leftover single
    # taps duplicated into both row halves for the K=64 row-group matmuls.
    dt_ = dic.transpose(1, 0, 2, 3)                       # [cin, d, kh, kw]
    wstk = np.zeros((128, 6 * DP), np.float32)
    for i, (ta, tb) in enumerate((((0, 0), (1, 1)),
                                  ((0, 1), (1, 2)),
                                  ((1, 0), (2, 1)))):
        wstk[0:64, i * DP:i * DP + D] = dt_[:, :, ta[0], ta[1]]
        wstk[64:128, i * DP:i * DP + D] = dt_[:, :, tb[0], tb[1]]
    for i, ts in enumerate(((0, 2), (2, 0), (2, 2))):
        wstk[0:64, (3 + i) * DP:(3 + i) * DP + D] = dt_[:, :, ts[0], ts[1]]
        wstk[64:128, (3 + i) * DP:(3 + i) * DP + D] = dt_[:, :, ts[0], ts[1]]
    wstk = wstk.astype(ml_dtypes.bfloat16)

    xpad = np.zeros((B, CIN, PH, PW), np.float32)
    xpad[:, :, 1:H + 1, 1:W + 1] = x
    xpad = xpad.astype(ml_dtypes.bfloat16)

    in_maps = []
    for c in range(NCORES):
        xf = xpad[c * BPC:(c + 1) * BPC].transpose(1, 0, 2, 3).reshape(CIN, F)
        xxk = np.zeros((128, FX), ml_dtypes.bfloat16)
        xxk[0:64, 0:F] = xf
        xxk[64:128, 0:F - PW - 1] = xf[:, PW + 1:]     # (+1 row, +1 col)
        in_maps.append({
            "xx": np.ascontiguousarray(xxk),
            "wst": wstk, "wm": wmp,
        })
    return in_maps


def _run(in_maps, trace=False, **kw):
    nc = _get_nc()
    return run_bass_kernel_spmd(nc, in_maps, core_ids=list(range(NCORES)),
                                trace=trace, **kw)


def kernel(x, dictionary, lookup_coefficients, lookup_indices):
    in_maps = _prep_inputs(x, dictionary, lookup_coefficients, lookup_indices)
    res = _run(in_maps)
    outs = [np.asarray(res.results[c]["out"]).astype(np.float32)
            for c in range(NCORES)]
    return np.concatenate(outs, axis=0)
